# revision 23
# baseline (speedup 1.0000x reference)
"""GroupedQueryAttention Trainium2 kernel (8 NeuronCores).

Sharding: (batch b in 0..1) x (kv-head group g in 0..3) -> core 4*b+g.
Each core computes, for its batch, the 4 query heads (4g..4g+3) that share
kv head g, plus the partial output projection through the matching 512-row
slice of Wo.  The host sums the 4 partials per batch.

On-device dataflow is fully "transposed": activations live as [feature,
token] so every matmul contraction sits on the partition axis, and the
softmax probabilities come out directly in the layout the P@V matmul
needs.  The whole computation is chunk-streamed (512 tokens per chunk):
projections -> rmsnorm+rope -> attention -> partial out-projection per
chunk, so all engines pipeline across chunks.

fp8 (e4m3, TRN flavor: max +-240) with DoubleRow matmuls is used for the
projections (x, Wq/Wk/Wv pre-packed host-side into contraction-pair
layout) and for P@V plus the softmax denominators (exp writes fp8 probs
directly; the denominator uses the same quantized probs so normalization
is consistent).  Scores (q.k) and the output projection stay bf16.

The causal mask is folded into the score PSUM accumulation as one extra
matmul (identity stationary x precomputed -30000 step tile) so exp
produces exact zeros above the diagonal and no post-exp masking op is
needed.  1/den is computed as exp(-ln(den)) on the scalar engine - both
functions live in the already-loaded activation table set.
"""

import numpy as np
import ml_dtypes

DIM, H, KV, S, B = 2048, 16, 4, 2048, 2
HD = DIM // H          # 128
GQ = H // KV           # 4 query heads per kv head
P = 128                # partitions
NCH = S // 512         # 4 sequence chunks of 512
NM = DIM // 256        # 8 double-row contraction tiles (256 features each)
EPS = 1e-6
BF = ml_dtypes.bfloat16
F8 = ml_dtypes.float8_e4m3   # TRN fp8_e4m3: max +-240

XSC = 16.0             # fp8 scale for x
WSC = 64.0             # fp8 scale for Wq/Wk/Wv
VSC = 32.0             # scale carried by v through P@V (folded out on host)
PSC = 16.0             # prob downscale inside exp (cancels in normalize)

_CACHED = {}


def _build_program():
    import concourse.bass as bass
    import concourse.tile as tile
    from concourse import bacc
    from concourse import mybir
    from concourse.masks import make_identity

    f32 = mybir.dt.float32
    bf16 = mybir.dt.bfloat16
    fp8 = mybir.dt.float8e4
    AF = mybir.ActivationFunctionType
    DR = mybir.MatmulPerfMode.DoubleRow

    nc = bacc.Bacc()
    # all parameters pre-packed on host so every DMA is contiguous per
    # partition
    xt = nc.declare_dram_parameter("xt", [P, NCH - 1, 16 * 512], fp8, isOutput=False)
    xt0 = nc.declare_dram_parameter("xt0", [P, 16 * 512], bf16, isOutput=False)
    wq = nc.declare_dram_parameter("wq", [P, 16, 512], fp8, isOutput=False)
    wk = nc.declare_dram_parameter("wk", [P, 16, HD], fp8, isOutput=False)
    wv = nc.declare_dram_parameter("wv", [P, 16, HD], fp8, isOutput=False)
    wqb = nc.declare_dram_parameter("wqb", [P, 16, 512], bf16, isOutput=False)
    wkb = nc.declare_dram_parameter("wkb", [P, 16, HD], bf16, isOutput=False)
    wvb = nc.declare_dram_parameter("wvb", [P, 16, HD], bf16, isOutput=False)
    wo = nc.declare_dram_parameter("wo", [P, GQ, DIM], bf16, isOutput=False)
    cosq = nc.declare_dram_parameter("cosq", [HD, S], bf16, isOutput=False)
    sinq = nc.declare_dram_parameter("sinq", [HD, S], bf16, isOutput=False)
    cosk = nc.declare_dram_parameter("cosk", [HD, S], bf16, isOutput=False)
    sink = nc.declare_dram_parameter("sink", [HD, S], bf16, isOutput=False)
    mneg = nc.declare_dram_parameter("mneg", [P, 4, 512], bf16, isOutput=False)
    rsw = nc.declare_dram_parameter("rsw", [P, P], bf16, isOutput=False)
    po = nc.declare_dram_parameter("po", [S, DIM], bf16, isOutput=True)

    inv_sqrt_hd = 1.0 / float(np.sqrt(HD))
    nln16 = -float(np.log(PSC))

    with tile.TileContext(nc) as tc:
      with tc.tile_pool(name="const", bufs=1) as const, \
           tc.tile_pool(name="wp", bufs=1) as wp, \
           tc.tile_pool(name="xtp", bufs=1) as xtp, \
           tc.tile_pool(name="actp", bufs=1) as actp, \
           tc.tile_pool(name="hatp", bufs=1) as hatp, \
           tc.tile_pool(name="w2", bufs=2) as w2, \
           tc.tile_pool(name="ptp", bufs=7) as ptp, \
           tc.tile_pool(name="recp", bufs=1) as recp, \
           tc.tile_pool(name="rowp", bufs=2) as rowp, \
           tc.tile_pool(name="ppP", bufs=2, space="PSUM") as ppP, \
           tc.tile_pool(name="psc", bufs=2, space="PSUM") as psc, \
           tc.tile_pool(name="pots", bufs=1, space="PSUM") as pots, \
           tc.tile_pool(name="pden", bufs=1, space="PSUM") as pden:

        ones_sb = const.tile([P, P], bf16)
        nc.vector.memset(ones_sb, 1.0)
        ones8 = const.tile([P, 2, P], fp8)
        nc.vector.memset(ones8, 1.0)
        ident = const.tile([P, P], bf16)
        make_identity(nc, ident)
        rsw_sb = const.tile([P, P], bf16)
        nc.scalar.dma_start(out=rsw_sb, in_=rsw[:, :])
        epsb = const.tile([P, 1], f32)
        nc.vector.memset(epsb, EPS)
        nl16b = const.tile([P, 1], f32)
        nc.vector.memset(nl16b, nln16)
        zerob = const.tile([P, 1], f32)
        nc.vector.memset(zerob, 0.0)

        # input staging.  sync queue, in dependency order: chunk-0 bf16
        # inputs first, then the fp8 pair-packed inputs for chunks 1-3.
        # gpsimd queue: the late-needed weights.
        wkb_sb = wp.tile([P, 16, HD], bf16)
        xt0_sb = xtp.tile([P, 16, 512], bf16)
        wqb_sb = wp.tile([P, 16, 512], bf16)
        wvb_sb = wp.tile([P, 16, HD], bf16)
        wk_sb = wp.tile([P, 16, HD], fp8)
        xt_sb = xtp.tile([P, NCH - 1, 16, 512], fp8)
        wq_sb = wp.tile([P, 16, 512], fp8)
        wv_sb = wp.tile([P, 16, HD], fp8)
        xt8_ap = xt.ap().rearrange("p c (m t) -> p c m t", m=16)
        nc.sync.dma_start(out=wkb_sb, in_=wkb.ap())
        nc.sync.dma_start(out=xt0_sb, in_=xt0.ap().rearrange("p (m t) -> p m t", m=16))
        nc.gpsimd.dma_start(out=xt_sb[:, 0], in_=xt8_ap[:, 0])
        nc.gpsimd.dma_start(out=wq_sb, in_=wq.ap())
        nc.gpsimd.dma_start(out=wk_sb, in_=wk.ap())
        nc.gpsimd.dma_start(out=wv_sb, in_=wv.ap())
        nc.gpsimd.dma_start(out=wqb_sb, in_=wqb.ap())
        nc.gpsimd.dma_start(out=wvb_sb, in_=wvb.ap())
        cs_tabs = {"cosq": cosq, "sinq": sinq, "cosk": cosk, "sink": sink}
        cs_sb = {}
        for nm in cs_tabs:
            cs_sb[nm] = wp.tile([P, S], bf16, tag=f"cs_{nm}", name=f"cs_{nm}")

        def fetch_cs(c):
            cl = slice(c * 512, (c + 1) * 512)
            for nm, t in cs_tabs.items():
                nc.scalar.dma_start(out=cs_sb[nm][:, cl], in_=t[:, cl])

        fetch_cs(0)
        mneg_sb = wp.tile([P, 4, 512], bf16)
        nc.scalar.dma_start(out=mneg_sb, in_=mneg.ap())
        wo_sb = wp.tile([P, GQ, DIM], bf16)

        # per-chunk rings for the pre-norm projections (slot 0-3 = q, 4 = k)
        src_t = {}
        vT = actp.tile([P, S], bf16, tag="vT")
        v_nat = actp.tile([P, 16, HD], fp8, tag="vnat")
        v_nat0 = actp.tile([P, 4, HD], bf16, tag="vnat0")
        qhat = [hatp.tile([P, S], bf16, tag=f"qhat{h}", name=f"qhat{h}")
                for h in range(GQ)]
        khat = hatp.tile([P, S], bf16, tag="khat")
        onorm = [hatp.tile([P, S], bf16, tag=f"onorm{h}", name=f"onorm{h}")
                 for h in range(GQ)]

        def emit_p1(c, slots=(4, 0, 1, 2, 3, 5)):
            # fp8 DoubleRow projections (transposed outputs)
            # slot order: k first (needed by every score tile), then q0-3, v
            sl = slice(c * 512, (c + 1) * 512)
            for slot in slots:
                ps = ppP.tile([P, 512], f32, tag="pp", name=f"proj_{c}_{slot}")
                if c == 0:
                    # chunk 0 in bf16: early tokens attend to few keys, so
                    # fp8 noise does not average out there
                    for j in range(16):
                        if slot < 4:
                            lhs = wqb_sb[:, j, slot * HD:(slot + 1) * HD]
                        elif slot == 4:
                            lhs = wkb_sb[:, j, :]
                        else:
                            lhs = wvb_sb[:, j, :]
                        nc.tensor.matmul(ps, lhs, xt0_sb[:, j, :],
                                         start=(j == 0), stop=(j == 15))
                else:
                    for m in range(NM):
                        if slot < 4:
                            lhs = wq_sb[:, 2 * m:2 * m + 2, slot * HD:(slot + 1) * HD]
                        elif slot == 4:
                            lhs = wk_sb[:, 2 * m:2 * m + 2, :]
                        else:
                            lhs = wv_sb[:, 2 * m:2 * m + 2, :]
                        nc.tensor.matmul(ps, lhs,
                                         xt_sb[:, c - 1, 2 * m:2 * m + 2, :],
                                         start=(m == 0), stop=(m == NM - 1),
                                         perf_mode=DR)
                vsc = VSC if c == 0 else (VSC / (XSC * WSC))
                if slot < 5:
                    st = actp.tile([P, 512], bf16, tag=f"src{slot}", bufs=2,
                                   name=f"src_{c}_{slot}")
                    src_t[(c, slot)] = st
                    nc.vector.tensor_copy(st, ps)
                else:
                    # store vT = VSC*v (chunk-0 PSUM is v, fp8 is XSC*WSC*v)
                    nc.scalar.activation(vT[:, sl], ps, AF.Copy, scale=vsc)
            if 5 not in slots:
                return
            # v -> natural layout [key_local, hd] for this chunk's 4 key tiles
            for jj in range(4):
                j = 4 * c + jj
                tp = ppP.tile([P, HD], bf16, tag="pp", name=f"vtr_{j}")
                nc.tensor.transpose(tp, vT[:, j * HD:(j + 1) * HD], ident)
                nc.scalar.copy(v_nat[:, j, :], tp)
                if c == 0:
                    nc.vector.tensor_copy(v_nat0[:, j, :], tp)

        def emit_p2(c):
            # rmsnorm (pre-gain) + rope, chunk-granular.  All 5 tensors'
            # square-sums are computed first so the PE block of matmuls runs
            # with its inputs already resolved; all 5 Ln's precede all 5
            # Exp's so the activation table swaps twice per chunk, not per
            # tensor; rope's rotation leaves PSUM immediately via an ACT
            # copy so the PSUM ring never waits on the DVE stream.
            sl = slice(c * 512, (c + 1) * 512)
            # late-needed inputs are issued from here so the early DMAs get
            # the full HBM bandwidth at kernel start
            if c + 1 < NCH:
                fetch_cs(c + 1)
            if c == 0:
                nc.scalar.dma_start(out=xt_sb[:, 1], in_=xt8_ap[:, 1])
                nc.scalar.dma_start(out=wo_sb, in_=wo.ap())
            elif c == 1:
                nc.scalar.dma_start(out=xt_sb[:, 2], in_=xt8_ap[:, 2])
            srcs = {t: src_t.pop((c, t)) for t in (4, 0, 1, 2, 3)}
            sqbs, rots, lnbs, t3s = {}, {}, {}, {}
            for t in (4, 0, 1, 2, 3):
                sqb = w2.tile([P, 512], bf16, tag="sqb", bufs=5)
                nc.gpsimd.tensor_mul(sqb, srcs[t], srcs[t])
                sqbs[t] = sqb
            for t in (4, 0, 1, 2, 3):
                ssq = ppP.tile([P, 512], f32, tag="pp", name=f"ssq_{c}_{t}")
                nc.tensor.matmul(ssq, ones_sb, sqbs[t], start=True, stop=True)
                lnb = w2.tile([P, 512], bf16, tag="lnb", bufs=5)
                nc.scalar.activation(lnb, ssq, AF.Ln, bias=epsb, scale=1.0 / HD)
                lnbs[t] = lnb
                rotp = ppP.tile([P, 512], f32, tag="pp", name=f"rot_{c}_{t}")
                nc.tensor.matmul(rotp, rsw_sb, srcs[t], start=True, stop=True)
                rot = w2.tile([P, 512], bf16, tag="rot", bufs=5)
                nc.scalar.copy(rot, rotp)
                rots[t] = rot
            for t in (4, 0, 1, 2, 3):
                cosT = cs_sb["cosq" if t < 4 else "cosk"][:, sl]
                sinT = cs_sb["sinq" if t < 4 else "sink"][:, sl]
                t1 = w2.tile([P, 512], bf16, tag="t1")
                nc.gpsimd.tensor_mul(t1, srcs[t], cosT)
                t2 = w2.tile([P, 512], bf16, tag="t2")
                nc.gpsimd.tensor_mul(t2, rots[t], sinT)
                t3 = w2.tile([P, 512], bf16, tag="t3", bufs=5)
                nc.gpsimd.tensor_add(t3, t1, t2)
                t3s[t] = t3
            rsbs = {}
            for t in (4, 0, 1, 2, 3):
                rsb = w2.tile([P, 512], bf16, tag="rsb", bufs=2)
                nc.scalar.activation(rsb, lnbs[t], AF.Exp, bias=zerob,
                                     scale=-0.5)
                rsbs[t] = rsb
            for t in (4, 0, 1, 2, 3):
                dst = (qhat[t] if t < 4 else khat)[:, sl]
                nc.gpsimd.tensor_mul(dst, t3s[t], rsbs[t])

        def emit_p3(c, p5of=None):
            # attention, head-outer.  Out-projection (i, n) groups of chunk
            # `p5of` (whose onorm settled a chunk ago) are interleaved
            # between score tiles: they fill the PE while the scalar engine
            # drains the exp queue, and never wait on the normalize chain.
            sl = slice(c * 512, (c + 1) * 512)
            npr = 2 * c + 2
            po_groups = ([(4 * p5of + i2, n) for i2 in range(4)
                          for n in range(NCH)] if p5of is not None else [])
            pgi = [0]

            def drain_po(k=1):
                while k > 0 and pgi[0] < len(po_groups):
                    emit_po_group(*po_groups[pgi[0]])
                    pgi[0] += 1
                    k -= 1

            for h in range(GQ):
                pts = []
                for pr in range(npr):
                    sc = psc.tile([P, 2, 512], f32, tag="sc",
                                  name=f"sc_{c}_{h}_{pr}")
                    for u in range(2):
                        j = 2 * pr + u
                        if j >= 4 * c:
                            # diagonal key tile: fold causal mask into the
                            # PSUM accumulation (-30000 above the diagonal)
                            nc.tensor.matmul(sc[:, u, :], ident,
                                             mneg_sb[:, j - 4 * c, :],
                                             start=True, stop=False)
                        nc.tensor.matmul(sc[:, u, :],
                                         khat[:, j * P:(j + 1) * P],
                                         qhat[h][:, sl],
                                         start=(j < 4 * c), stop=True)
                    if c == 0:
                        pt = ptp.tile([P, 2, 512], bf16, tag="pt0",
                                      name=f"pt_{c}_{h}_{pr}", bufs=2)
                    else:
                        pt = ptp.tile([P, 2, 512], fp8, tag="pt",
                                      name=f"pt_{c}_{h}_{pr}")
                    nc.scalar.activation(pt, sc, AF.Exp,
                                         bias=nl16b, scale=inv_sqrt_hd)
                    pts.append(pt)
                    if pr % 2 == 1:
                        drain_po()
                ot = pots.tile([P, 512], f32, tag="ot", name=f"ot_{c}_{h}")
                dn = pden.tile([P, 512], f32, tag="dn", name=f"dn_{c}_{h}")
                if c == 0:
                    # chunk 0 stays bf16 through P@V / denominator as well
                    for pr in range(npr):
                        for u in range(2):
                            j = 2 * pr + u
                            nc.tensor.matmul(ot, v_nat0[:, j, :],
                                             pts[pr][:, u, :],
                                             start=(j == 0), stop=(j == 3))
                    for pr in range(npr):
                        for u in range(2):
                            j = 2 * pr + u
                            nc.tensor.matmul(dn, ones_sb, pts[pr][:, u, :],
                                             start=(j == 0), stop=(j == 3))
                else:
                    for pr in range(npr):
                        nc.tensor.matmul(ot, v_nat[:, 2 * pr:2 * pr + 2, :],
                                         pts[pr], start=(pr == 0),
                                         stop=(pr == npr - 1), perf_mode=DR)
                    for pr in range(npr):
                        nc.tensor.matmul(dn, ones8, pts[pr], start=(pr == 0),
                                         stop=(pr == npr - 1), perf_mode=DR)
                rec = recp.tile([P, 512], f32, tag="rec")
                nc.vector.reciprocal(rec, dn)
                nc.vector.tensor_mul(onorm[h][:, sl], ot, rec)
                drain_po()
            drain_po(len(po_groups))

        def emit_po_group(i, n):
            # one 128-token x 512-col group of the partial out-projection
            isl = slice(i * P, (i + 1) * P)
            pops = ppP.tile([P, 512], f32, tag="pp", name=f"po_{i}_{n}")
            for h in range(GQ):
                nc.tensor.matmul(pops, onorm[h][:, isl],
                                 wo_sb[:, h, n * 512:(n + 1) * 512],
                                 start=(h == 0), stop=(h == GQ - 1))
            row = rowp.tile([P, 512], bf16, tag="row")
            nc.vector.tensor_copy(row, pops)
            nc.sync.dma_start(out=po[isl, n * 512:(n + 1) * 512],
                              in_=row)

        # emission order tuned for the in-order engine queues: the chunk-0
        # k-projection (smallest DMA footprint) first, then the fp8 chunk-1
        # projections (their inputs are small and land early), then the rest
        # of chunk 0.  P5(c) is emitted interleaved into P3(c+1) a chunk
        # late so the normalize chain's latency is fully hidden.
        emit_p1(0, slots=(4,))
        emit_p1(1)
        emit_p1(0, slots=(0, 1, 2, 3, 5))
        emit_p2(0)
        emit_p3(0)
        emit_p1(2)
        emit_p2(1)
        emit_p3(1, p5of=0)
        emit_p1(3)
        emit_p2(2)
        emit_p3(2, p5of=1)
        emit_p2(3)
        emit_p3(3, p5of=2)
        for i in range(12, 16):
            for n in range(NCH):
                emit_po_group(i, n)
    nc.compile()
    return nc


def _causal_ok(mask):
    m = np.asarray(mask).reshape(S, S)
    tri = np.tril(np.ones((S, S), dtype=bool))
    return bool(np.all(m[tri] == 0.0) and np.all(m[~tri] <= -1e8))


def _reference_fallback(x, Wq, Wk, Wv, Wo, qg, kg, cos, sin, mask):
    x64 = np.asarray(x, dtype=np.float32)
    q = (x64 @ Wq).reshape(B, S, H, HD).transpose(0, 2, 1, 3)
    k = (x64 @ Wk).reshape(B, S, KV, HD).transpose(0, 2, 1, 3)
    v = (x64 @ Wv).reshape(B, S, KV, HD).transpose(0, 2, 1, 3)

    def rms(t, g):
        r = np.sqrt(np.mean(t * t, axis=-1, keepdims=True) + EPS)
        return g * (t / r)

    q, k = rms(q, qg), rms(k, kg)

    def rot(t):
        return np.concatenate([-t[..., HD // 2:], t[..., :HD // 2]], axis=-1)

    c = cos[None, None, :, :]
    s = sin[None, None, :, :]
    q = q * c + rot(q) * s
    k = k * c + rot(k) * s
    k = np.repeat(k, GQ, axis=1)
    v = np.repeat(v, GQ, axis=1)
    sc = np.einsum('bhqd,bhkd->bhqk', q, k) / np.sqrt(HD) + np.asarray(mask).reshape(1, 1, S, S)
    sc = sc - sc.max(axis=-1, keepdims=True)
    e = np.exp(sc)
    a = e / e.sum(axis=-1, keepdims=True)
    o = np.einsum('bhqk,bhkd->bhqd', a, v)
    o = o.transpose(0, 2, 1, 3).reshape(B, S, H * HD)
    return (o @ Wo).astype(np.float32)


def _to_f8(a):
    return np.ascontiguousarray(np.clip(a, -240.0, 240.0)).astype(F8)


def _pack_dr(w):
    """[DIM, cols] -> [P, 2*m, cols] with feature f = m*256 + u*128 + p."""
    cols = w.shape[1]
    return np.ascontiguousarray(
        w.reshape(NM, 2, P, cols).transpose(2, 0, 1, 3).reshape(P, 2 * NM, cols))


def _pack_bf(w):
    """[DIM, cols] -> [P, j, cols] bf16 with feature f = j*128 + p."""
    cols = w.shape[1]
    return np.ascontiguousarray(
        w.reshape(16, P, cols).transpose(1, 0, 2)).astype(BF)


def kernel(x, Wq, Wk, Wv, Wo, qg, kg, cos, sin, mask, **_unused):
    x = np.asarray(x, dtype=np.float32)
    Wq, Wk, Wv, Wo = (np.asarray(a, dtype=np.float32) for a in (Wq, Wk, Wv, Wo))
    qg, kg = np.asarray(qg, np.float32), np.asarray(kg, np.float32)
    cos, sin = np.asarray(cos, np.float32), np.asarray(sin, np.float32)
    if not _causal_ok(mask):
        return _reference_fallback(x, Wq, Wk, Wv, Wo, qg, kg, cos, sin, mask)

    from concourse.bass_utils import run_bass_kernel_spmd

    if "nc" not in _CACHED:
        _CACHED["nc"] = _build_program()
    nc = _CACHED["nc"]

    cosT = np.ascontiguousarray(cos.T)  # [HD, S]
    sinT = np.ascontiguousarray(sin.T)

    # rope via halves: out[:64] = x[:64]*cos[:64] + x[64:]*st[:64]
    #                  out[64:] = x[64:]*cos[64:] + x[:64]*st[64:]
    # (rot(x)[:64] = -x[64:], rot(x)[64:] = x[:64]; gains fold into tables)
    def tables(g):
        ct = cosT * g[:, None]
        st = np.empty_like(sinT)
        st[:64] = -sinT[:64] * g[64:, None]
        st[64:] = sinT[64:] * g[:64, None]
        return ct.astype(BF), st.astype(BF)

    cq, sq = tables(qg)
    ck, sk = tables(kg)

    rsw = np.zeros((P, P), dtype=np.float32)
    for i in range(P):
        rsw[i, (i + 64) % P] = 1.0
    rsw = rsw.astype(BF)

    cols = np.arange(512)[None, :]
    rows = np.arange(P)[:, None]
    mneg = np.stack([
        np.where(cols - P * a >= rows, 0.0, -30000.0) for a in range(4)
    ]).transpose(1, 0, 2).astype(BF)          # [P, 4, 512]
    mneg = np.ascontiguousarray(mneg)

    # xt: [P, chunk-1, m, u, t] fp8 (chunks 1-3), f = m*256 + u*128 + p
    # xt0: [P, j, t] bf16 (chunk 0), f = j*128 + p
    xts, xt0s = [], []
    for b in range(B):
        xr = x[b].T.reshape(NM, 2, P, NCH, 512)       # [m,u,p,c,t]
        xr = xr.transpose(2, 3, 0, 1, 4).reshape(P, NCH, 16 * 512)
        xts.append(_to_f8(xr[:, 1:] * XSC))
        x0 = x[b][:512].T.reshape(16, P, 512).transpose(1, 0, 2)  # [p,j,t]
        xt0s.append(np.ascontiguousarray(x0.reshape(P, 16 * 512)).astype(BF))

    in_maps = []
    for core in range(8):
        b, g = divmod(core, KV)
        wq_g = Wq[:, g * GQ * HD:(g + 1) * GQ * HD]
        wk_g = Wk[:, g * HD:(g + 1) * HD]
        wv_g = Wv[:, g * HD:(g + 1) * HD]
        in_maps.append({
            "xt": xts[b], "xt0": xt0s[b],
            "wq": _to_f8(_pack_dr(wq_g) * WSC),
            "wk": _to_f8(_pack_dr(wk_g) * WSC),
            "wv": _to_f8(_pack_dr(wv_g) * WSC),
            "wqb": _pack_bf(wq_g), "wkb": _pack_bf(wk_g), "wvb": _pack_bf(wv_g),
            "wo": np.ascontiguousarray(
                Wo[g * GQ * HD:(g + 1) * GQ * HD, :].reshape(GQ, P, DIM)
                .transpose(1, 0, 2)).astype(BF),
            "cosq": cq, "sinq": sq, "cosk": ck, "sink": sk,
            "mneg": mneg, "rsw": rsw,
        })

    res = run_bass_kernel_spmd(nc, in_maps, list(range(8)))
    out = np.zeros((B, S, DIM), dtype=np.float32)
    for core in range(8):
        out[core // KV] += res.results[core]["po"].astype(np.float32)
    out *= 1.0 / VSC
    return out


# revision 30
# speedup vs baseline: 1.1254x; 1.1254x over previous
"""GroupedQueryAttention Trainium2 kernel (8 NeuronCores).

Sharding: (batch b in 0..1) x (kv-head group g in 0..3) -> core 4*b+g.
Each core computes, for its batch, the 4 query heads (4g..4g+3) that share
kv head g, plus the partial output projection through the matching 512-row
slice of Wo.  The host sums the 4 partials per batch.

On-device dataflow is fully "transposed": activations live as [feature,
token] so every matmul contraction sits on the partition axis, and the
softmax probabilities come out directly in the layout the P@V matmul
needs.  The whole computation is chunk-streamed (512 tokens per chunk):
projections -> rmsnorm+rope -> attention -> partial out-projection per
chunk, so all engines pipeline across chunks.

fp8 (e4m3, TRN flavor: max +-240) with DoubleRow matmuls is used for the
projections (x, Wq/Wk/Wv pre-packed host-side into contraction-pair
layout) and for P@V plus the softmax denominators (exp writes fp8 probs
directly; the denominator uses the same quantized probs so normalization
is consistent).  Scores (q.k) and the output projection stay bf16.

The causal mask is folded into the score PSUM accumulation as one extra
matmul (identity stationary x precomputed -30000 step tile) so exp
produces exact zeros above the diagonal and no post-exp masking op is
needed.  1/den is computed as exp(-ln(den)) on the scalar engine - both
functions live in the already-loaded activation table set.
"""

import numpy as np
import ml_dtypes

DIM, H, KV, S, B = 2048, 16, 4, 2048, 2
HD = DIM // H          # 128
GQ = H // KV           # 4 query heads per kv head
P = 128                # partitions
NCH = S // 512         # 4 sequence chunks of 512
NM = DIM // 256        # 8 double-row contraction tiles (256 features each)
EPS = 1e-6
BF = ml_dtypes.bfloat16
F8 = ml_dtypes.float8_e4m3   # TRN fp8_e4m3: max +-240

XSC = 16.0             # fp8 scale for x
WSC = 64.0             # fp8 scale for Wq/Wk/Wv
VSC = 32.0             # scale carried by v through P@V (folded out on host)
PSC = 16.0             # prob downscale inside exp (cancels in normalize)

_CACHED = {}


def _build_program():
    import concourse.bass as bass
    import concourse.tile as tile
    from concourse import bacc
    from concourse import mybir
    from concourse.masks import make_identity

    f32 = mybir.dt.float32
    bf16 = mybir.dt.bfloat16
    fp8 = mybir.dt.float8e4
    AF = mybir.ActivationFunctionType
    DR = mybir.MatmulPerfMode.DoubleRow

    nc = bacc.Bacc()
    # all parameters pre-packed on host so every DMA is contiguous per
    # partition
    xt = nc.declare_dram_parameter("xt", [P, NCH - 1, 16 * 512], fp8, isOutput=False)
    xt0 = nc.declare_dram_parameter("xt0", [P, 16 * 512], bf16, isOutput=False)
    wq = nc.declare_dram_parameter("wq", [P, 16, 512], fp8, isOutput=False)
    wk = nc.declare_dram_parameter("wk", [P, 16, HD], fp8, isOutput=False)
    wv = nc.declare_dram_parameter("wv", [P, 16, HD], fp8, isOutput=False)
    wqb = nc.declare_dram_parameter("wqb", [P, 16, 512], bf16, isOutput=False)
    wkb = nc.declare_dram_parameter("wkb", [P, 16, HD], bf16, isOutput=False)
    wvb = nc.declare_dram_parameter("wvb", [P, 16, HD], bf16, isOutput=False)
    wo = nc.declare_dram_parameter("wo", [P, GQ, DIM], bf16, isOutput=False)
    cosq = nc.declare_dram_parameter("cosq", [HD, S], bf16, isOutput=False)
    sinq = nc.declare_dram_parameter("sinq", [HD, S], bf16, isOutput=False)
    cosk = nc.declare_dram_parameter("cosk", [HD, S], bf16, isOutput=False)
    sink = nc.declare_dram_parameter("sink", [HD, S], bf16, isOutput=False)
    mneg = nc.declare_dram_parameter("mneg", [P, 4, 512], bf16, isOutput=False)
    rsw = nc.declare_dram_parameter("rsw", [P, P], bf16, isOutput=False)
    po = nc.declare_dram_parameter("po", [S, DIM], bf16, isOutput=True)

    inv_sqrt_hd = 1.0 / float(np.sqrt(HD))
    nln16 = -float(np.log(PSC))

    with tile.TileContext(nc) as tc:
      with tc.tile_pool(name="const", bufs=1) as const, \
           tc.tile_pool(name="wp", bufs=1) as wp, \
           tc.tile_pool(name="xtp", bufs=1) as xtp, \
           tc.tile_pool(name="actp", bufs=1) as actp, \
           tc.tile_pool(name="hatp", bufs=1) as hatp, \
           tc.tile_pool(name="w2", bufs=2) as w2, \
           tc.tile_pool(name="ptp", bufs=7) as ptp, \
           tc.tile_pool(name="rowp", bufs=2) as rowp, \
           tc.tile_pool(name="ppP", bufs=2, space="PSUM") as ppP, \
           tc.tile_pool(name="psc", bufs=2, space="PSUM") as psc, \
           tc.tile_pool(name="pots", bufs=1, space="PSUM") as pots, \
           tc.tile_pool(name="pden", bufs=1, space="PSUM") as pden:

        ones_sb = const.tile([P, P], bf16)
        nc.vector.memset(ones_sb, 1.0)
        ones8 = const.tile([P, 2, P], fp8)
        nc.vector.memset(ones8, 1.0)
        ident = const.tile([P, P], bf16)
        make_identity(nc, ident)
        rsw_sb = const.tile([P, P], bf16)
        nc.scalar.dma_start(out=rsw_sb, in_=rsw[:, :])
        epsb = const.tile([P, 1], f32)
        nc.vector.memset(epsb, EPS)
        nl16b = const.tile([P, 1], f32)
        nc.vector.memset(nl16b, nln16)
        zerob = const.tile([P, 1], f32)
        nc.vector.memset(zerob, 0.0)

        # input staging.  sync queue, in dependency order: chunk-0 bf16
        # inputs first, then the fp8 pair-packed inputs for chunks 1-3.
        # gpsimd queue: the late-needed weights.
        wkb_sb = wp.tile([P, 16, HD], bf16)
        xt0_sb = xtp.tile([P, 16, 512], bf16)
        wqb_sb = wp.tile([P, 16, 512], bf16)
        wvb_sb = wp.tile([P, 16, HD], bf16)
        wk_sb = wp.tile([P, 16, HD], fp8)
        xt_sb = xtp.tile([P, NCH - 1, 16, 512], fp8)
        wq_sb = wp.tile([P, 16, 512], fp8)
        wv_sb = wp.tile([P, 16, HD], fp8)
        xt8_ap = xt.ap().rearrange("p c (m t) -> p c m t", m=16)
        nc.sync.dma_start(out=wkb_sb, in_=wkb.ap())
        nc.sync.dma_start(out=xt0_sb, in_=xt0.ap().rearrange("p (m t) -> p m t", m=16))
        # gate the second DMA wave on the chunk-0 inputs having landed so
        # the startup-critical transfers get the full HBM bandwidth
        scr = const.tile([P, 16], bf16)
        nc.gpsimd.tensor_copy(scr, xt0_sb[:, 0, 0:16])
        nc.gpsimd.dma_start(out=xt_sb[:, 0], in_=xt8_ap[:, 0])
        nc.gpsimd.dma_start(out=wq_sb, in_=wq.ap())
        nc.gpsimd.dma_start(out=wk_sb, in_=wk.ap())
        nc.gpsimd.dma_start(out=wv_sb, in_=wv.ap())
        nc.gpsimd.dma_start(out=wqb_sb, in_=wqb.ap())
        nc.gpsimd.dma_start(out=wvb_sb, in_=wvb.ap())
        cs_tabs = {"cosq": cosq, "sinq": sinq, "cosk": cosk, "sink": sink}
        cs_sb = {}
        for nm in cs_tabs:
            cs_sb[nm] = wp.tile([P, S], bf16, tag=f"cs_{nm}", name=f"cs_{nm}")

        def fetch_cs(c):
            cl = slice(c * 512, (c + 1) * 512)
            for nm, t in cs_tabs.items():
                nc.scalar.dma_start(out=cs_sb[nm][:, cl], in_=t[:, cl])

        fetch_cs(0)
        mneg_sb = wp.tile([P, 4, 512], bf16)
        nc.scalar.dma_start(out=mneg_sb, in_=mneg.ap())
        wo_sb = wp.tile([P, GQ, DIM], bf16)

        # per-chunk rings for the pre-norm projections (slot 0-3 = q, 4 = k)
        src_t = {}
        vT = actp.tile([P, S], bf16, tag="vT")
        ot_t = {}
        dn_t = {}
        onorm_t = {}
        v_nat = actp.tile([P, 16, HD], fp8, tag="vnat")
        v_nat0 = actp.tile([P, 4, HD], bf16, tag="vnat0")
        qhat = [hatp.tile([P, S], bf16, tag=f"qhat{h}", name=f"qhat{h}")
                for h in range(GQ)]
        khat = hatp.tile([P, S], bf16, tag="khat")

        def emit_p1(c, slots=(4, 0, 1, 2, 3, 5)):
            # fp8 DoubleRow projections (transposed outputs)
            # slot order: k first (needed by every score tile), then q0-3, v
            sl = slice(c * 512, (c + 1) * 512)
            for slot in slots:
                ps = ppP.tile([P, 512], f32, tag="pp", name=f"proj_{c}_{slot}")
                if c == 0:
                    # chunk 0 in bf16: early tokens attend to few keys, so
                    # fp8 noise does not average out there
                    for j in range(16):
                        if slot < 4:
                            lhs = wqb_sb[:, j, slot * HD:(slot + 1) * HD]
                        elif slot == 4:
                            lhs = wkb_sb[:, j, :]
                        else:
                            lhs = wvb_sb[:, j, :]
                        nc.tensor.matmul(ps, lhs, xt0_sb[:, j, :],
                                         start=(j == 0), stop=(j == 15))
                else:
                    for m in range(NM):
                        if slot < 4:
                            lhs = wq_sb[:, 2 * m:2 * m + 2, slot * HD:(slot + 1) * HD]
                        elif slot == 4:
                            lhs = wk_sb[:, 2 * m:2 * m + 2, :]
                        else:
                            lhs = wv_sb[:, 2 * m:2 * m + 2, :]
                        nc.tensor.matmul(ps, lhs,
                                         xt_sb[:, c - 1, 2 * m:2 * m + 2, :],
                                         start=(m == 0), stop=(m == NM - 1),
                                         perf_mode=DR)
                vsc = VSC if c == 0 else (VSC / (XSC * WSC))
                if slot < 5:
                    st = actp.tile([P, 512], bf16, tag=f"src{slot}", bufs=2,
                                   name=f"src_{c}_{slot}")
                    src_t[(c, slot)] = st
                    nc.vector.tensor_copy(st, ps)
                else:
                    # store vT = VSC*v (chunk-0 PSUM is v, fp8 is XSC*WSC*v)
                    nc.scalar.activation(vT[:, sl], ps, AF.Copy, scale=vsc)
            if 5 not in slots:
                return
            # v -> natural layout [key_local, hd] for this chunk's 4 key tiles
            for jj in range(4):
                j = 4 * c + jj
                tp = ppP.tile([P, HD], bf16, tag="pp", name=f"vtr_{j}")
                nc.tensor.transpose(tp, vT[:, j * HD:(j + 1) * HD], ident)
                nc.scalar.copy(v_nat[:, j, :], tp)
                if c == 0:
                    nc.vector.tensor_copy(v_nat0[:, j, :], tp)

        def emit_p2(c):
            # rmsnorm (pre-gain) + rope, chunk-granular.  All 5 tensors'
            # square-sums are computed first so the PE block of matmuls runs
            # with its inputs already resolved; all 5 Ln's precede all 5
            # Exp's so the activation table swaps twice per chunk, not per
            # tensor; rope's rotation leaves PSUM immediately via an ACT
            # copy so the PSUM ring never waits on the DVE stream.
            sl = slice(c * 512, (c + 1) * 512)
            # late-needed inputs are issued from here so the early DMAs get
            # the full HBM bandwidth at kernel start
            if c + 1 < NCH:
                fetch_cs(c + 1)
            if c == 0:
                nc.scalar.dma_start(out=xt_sb[:, 1], in_=xt8_ap[:, 1])
                nc.scalar.dma_start(out=wo_sb, in_=wo.ap())
            elif c == 1:
                nc.scalar.dma_start(out=xt_sb[:, 2], in_=xt8_ap[:, 2])
            srcs = {t: src_t.pop((c, t)) for t in (4, 0, 1, 2, 3)}
            sqbs, rots, lnbs, t3s = {}, {}, {}, {}
            for t in (4, 0, 1, 2, 3):
                sqb = w2.tile([P, 512], bf16, tag="sqb", bufs=5)
                nc.vector.tensor_mul(sqb, srcs[t], srcs[t])
                sqbs[t] = sqb
            # --- Ln block: rmsnorm logs + previous chunk's denominator logs
            # share one activation-table residency ---
            for t in (4, 0, 1, 2, 3):
                ssq = ppP.tile([P, 512], f32, tag="pp", name=f"ssq_{c}_{t}")
                nc.tensor.matmul(ssq, ones_sb, sqbs[t], start=True, stop=True)
                lnb = w2.tile([P, 512], bf16, tag="lnb", bufs=5)
                nc.scalar.activation(lnb, ssq, AF.Ln, bias=epsb, scale=1.0 / HD)
                lnbs[t] = lnb
                rotp = ppP.tile([P, 512], f32, tag="pp", name=f"rot_{c}_{t}")
                nc.tensor.matmul(rotp, rsw_sb, srcs[t], start=True, stop=True)
                rot = w2.tile([P, 512], bf16, tag="rot", bufs=5)
                nc.scalar.copy(rot, rotp)
                rots[t] = rot
            for t in (4, 0, 1, 2, 3):
                cosT = cs_sb["cosq" if t < 4 else "cosk"][:, sl]
                sinT = cs_sb["sinq" if t < 4 else "sink"][:, sl]
                t1 = w2.tile([P, 512], bf16, tag="t1")
                nc.vector.tensor_mul(t1, srcs[t], cosT)
                t2 = w2.tile([P, 512], bf16, tag="t2")
                nc.vector.tensor_mul(t2, rots[t], sinT)
                t3 = w2.tile([P, 512], bf16, tag="t3", bufs=5)
                nc.vector.tensor_add(t3, t1, t2)
                t3s[t] = t3
            # --- Exp block ---
            rsbs = {}
            for t in (4, 0, 1, 2, 3):
                rsb = w2.tile([P, 512], bf16, tag="rsb", bufs=2)
                nc.scalar.activation(rsb, lnbs[t], AF.Exp, bias=zerob,
                                     scale=-0.5)
                rsbs[t] = rsb
            for t in (4, 0, 1, 2, 3):
                dst = (qhat[t] if t < 4 else khat)[:, sl]
                nc.vector.tensor_mul(dst, t3s[t], rsbs[t])


        def emit_p3(c, p5of=None):
            # attention, head-outer.  Out-projection (i, n) groups of chunk
            # `p5of` (whose onorm settled a chunk ago) are interleaved
            # between score tiles: they fill the PE while the scalar engine
            # drains the exp queue, and never wait on the normalize chain.
            sl = slice(c * 512, (c + 1) * 512)
            npr = 2 * c + 2
            po_groups = ([(4 * p5of + i2, n) for i2 in range(4)
                          for n in range(NCH)] if p5of is not None else [])
            pgi = [0]

            def drain_po(k=1):
                while k > 0 and pgi[0] < len(po_groups):
                    emit_po_group(*po_groups[pgi[0]])
                    pgi[0] += 1
                    k -= 1

            for h in range(GQ):
                pts = []
                for pr in range(npr):
                    sc = psc.tile([P, 2, 512], f32, tag="sc",
                                  name=f"sc_{c}_{h}_{pr}")
                    for u in range(2):
                        j = 2 * pr + u
                        if j >= 4 * c:
                            # diagonal key tile: fold causal mask into the
                            # PSUM accumulation (-30000 above the diagonal)
                            nc.tensor.matmul(sc[:, u, :], ident,
                                             mneg_sb[:, j - 4 * c, :],
                                             start=True, stop=False)
                        nc.tensor.matmul(sc[:, u, :],
                                         khat[:, j * P:(j + 1) * P],
                                         qhat[h][:, sl],
                                         start=(j < 4 * c), stop=True)
                    if c == 0:
                        pt = ptp.tile([P, 2, 512], bf16, tag="pt0",
                                      name=f"pt_{c}_{h}_{pr}", bufs=2)
                    else:
                        pt = ptp.tile([P, 2, 512], fp8, tag="pt",
                                      name=f"pt_{c}_{h}_{pr}")
                    nc.scalar.activation(pt, sc, AF.Exp,
                                         bias=nl16b, scale=inv_sqrt_hd)
                    pts.append(pt)
                    if pr % 2 == 1:
                        drain_po()
                ot = pots.tile([P, 512], f32, tag="ot", name=f"ot_{c}_{h}")
                dn = pden.tile([P, 512], f32, tag="dn", name=f"dn_{c}_{h}")
                if c == 0:
                    # chunk 0 stays bf16 through P@V / denominator as well
                    for pr in range(npr):
                        for u in range(2):
                            j = 2 * pr + u
                            nc.tensor.matmul(ot, v_nat0[:, j, :],
                                             pts[pr][:, u, :],
                                             start=(j == 0), stop=(j == 3))
                    for pr in range(npr):
                        for u in range(2):
                            j = 2 * pr + u
                            nc.tensor.matmul(dn, ones_sb, pts[pr][:, u, :],
                                             start=(j == 0), stop=(j == 3))
                else:
                    for pr in range(npr):
                        nc.tensor.matmul(ot, v_nat[:, 2 * pr:2 * pr + 2, :],
                                         pts[pr], start=(pr == 0),
                                         stop=(pr == npr - 1), perf_mode=DR)
                    for pr in range(npr):
                        nc.tensor.matmul(dn, ones8, pts[pr], start=(pr == 0),
                                         stop=(pr == npr - 1), perf_mode=DR)
                rec = w2.tile([P, 512], f32, tag="rec", bufs=2)
                nc.vector.reciprocal(rec, dn)
                ono = hatp.tile([P, 512], bf16, tag=f"onorm{h}", bufs=2,
                                name=f"onorm_{c}_{h}")
                onorm_t[(c, h)] = ono
                nc.vector.tensor_mul(ono, ot, rec)
                drain_po()
            drain_po(len(po_groups))

        def emit_po_group(i, n):
            # one 128-token x 512-col group of the partial out-projection
            ci, i2 = divmod(i, 4)
            isl = slice(i2 * P, (i2 + 1) * P)
            osl = slice(i * P, (i + 1) * P)
            pops = ppP.tile([P, 512], f32, tag="pp", name=f"po_{i}_{n}")
            for h in range(GQ):
                nc.tensor.matmul(pops, onorm_t[(ci, h)][:, isl],
                                 wo_sb[:, h, n * 512:(n + 1) * 512],
                                 start=(h == 0), stop=(h == GQ - 1))
            row = rowp.tile([P, 512], bf16, tag="row")
            nc.vector.tensor_copy(row, pops)
            nc.sync.dma_start(out=po[osl, n * 512:(n + 1) * 512],
                              in_=row)

        # emission order tuned for the in-order engine queues: the chunk-0
        # k-projection (smallest DMA footprint) first, then the fp8 chunk-1
        # projections (their inputs are small and land early), then the rest
        # of chunk 0.  P5(c) is emitted interleaved into P3(c+1) a chunk
        # late so the normalize chain's latency is fully hidden.
        emit_p1(0, slots=(4,))
        emit_p1(1)
        emit_p1(0, slots=(0, 1, 2, 3, 5))
        emit_p2(0)
        emit_p3(0)
        emit_p1(2)
        emit_p2(1)
        emit_p3(1, p5of=0)
        emit_p1(3)
        emit_p2(2)
        emit_p3(2, p5of=1)
        emit_p2(3)
        emit_p3(3, p5of=2)
        for i in range(12, 16):
            for n in range(NCH):
                emit_po_group(i, n)
    nc.compile()
    return nc


def _causal_ok(mask):
    m = np.asarray(mask).reshape(S, S)
    tri = np.tril(np.ones((S, S), dtype=bool))
    return bool(np.all(m[tri] == 0.0) and np.all(m[~tri] <= -1e8))


def _reference_fallback(x, Wq, Wk, Wv, Wo, qg, kg, cos, sin, mask):
    x64 = np.asarray(x, dtype=np.float32)
    q = (x64 @ Wq).reshape(B, S, H, HD).transpose(0, 2, 1, 3)
    k = (x64 @ Wk).reshape(B, S, KV, HD).transpose(0, 2, 1, 3)
    v = (x64 @ Wv).reshape(B, S, KV, HD).transpose(0, 2, 1, 3)

    def rms(t, g):
        r = np.sqrt(np.mean(t * t, axis=-1, keepdims=True) + EPS)
        return g * (t / r)

    q, k = rms(q, qg), rms(k, kg)

    def rot(t):
        return np.concatenate([-t[..., HD // 2:], t[..., :HD // 2]], axis=-1)

    c = cos[None, None, :, :]
    s = sin[None, None, :, :]
    q = q * c + rot(q) * s
    k = k * c + rot(k) * s
    k = np.repeat(k, GQ, axis=1)
    v = np.repeat(v, GQ, axis=1)
    sc = np.einsum('bhqd,bhkd->bhqk', q, k) / np.sqrt(HD) + np.asarray(mask).reshape(1, 1, S, S)
    sc = sc - sc.max(axis=-1, keepdims=True)
    e = np.exp(sc)
    a = e / e.sum(axis=-1, keepdims=True)
    o = np.einsum('bhqk,bhkd->bhqd', a, v)
    o = o.transpose(0, 2, 1, 3).reshape(B, S, H * HD)
    return (o @ Wo).astype(np.float32)


def _to_f8(a):
    return np.ascontiguousarray(np.clip(a, -240.0, 240.0)).astype(F8)


def _pack_dr(w):
    """[DIM, cols] -> [P, 2*m, cols] with feature f = m*256 + u*128 + p."""
    cols = w.shape[1]
    return np.ascontiguousarray(
        w.reshape(NM, 2, P, cols).transpose(2, 0, 1, 3).reshape(P, 2 * NM, cols))


def _pack_bf(w):
    """[DIM, cols] -> [P, j, cols] bf16 with feature f = j*128 + p."""
    cols = w.shape[1]
    return np.ascontiguousarray(
        w.reshape(16, P, cols).transpose(1, 0, 2)).astype(BF)


def kernel(x, Wq, Wk, Wv, Wo, qg, kg, cos, sin, mask, **_unused):
    x = np.asarray(x, dtype=np.float32)
    Wq, Wk, Wv, Wo = (np.asarray(a, dtype=np.float32) for a in (Wq, Wk, Wv, Wo))
    qg, kg = np.asarray(qg, np.float32), np.asarray(kg, np.float32)
    cos, sin = np.asarray(cos, np.float32), np.asarray(sin, np.float32)
    if not _causal_ok(mask):
        return _reference_fallback(x, Wq, Wk, Wv, Wo, qg, kg, cos, sin, mask)

    from concourse.bass_utils import run_bass_kernel_spmd

    if "nc" not in _CACHED:
        _CACHED["nc"] = _build_program()
    nc = _CACHED["nc"]

    cosT = np.ascontiguousarray(cos.T)  # [HD, S]
    sinT = np.ascontiguousarray(sin.T)

    # rope via halves: out[:64] = x[:64]*cos[:64] + x[64:]*st[:64]
    #                  out[64:] = x[64:]*cos[64:] + x[:64]*st[64:]
    # (rot(x)[:64] = -x[64:], rot(x)[64:] = x[:64]; gains fold into tables)
    def tables(g):
        ct = cosT * g[:, None]
        st = np.empty_like(sinT)
        st[:64] = -sinT[:64] * g[64:, None]
        st[64:] = sinT[64:] * g[:64, None]
        return ct.astype(BF), st.astype(BF)

    cq, sq = tables(qg)
    ck, sk = tables(kg)

    rsw = np.zeros((P, P), dtype=np.float32)
    for i in range(P):
        rsw[i, (i + 64) % P] = 1.0
    rsw = rsw.astype(BF)

    cols = np.arange(512)[None, :]
    rows = np.arange(P)[:, None]
    mneg = np.stack([
        np.where(cols - P * a >= rows, 0.0, -30000.0) for a in range(4)
    ]).transpose(1, 0, 2).astype(BF)          # [P, 4, 512]
    mneg = np.ascontiguousarray(mneg)

    # xt: [P, chunk-1, m, u, t] fp8 (chunks 1-3), f = m*256 + u*128 + p
    # xt0: [P, j, t] bf16 (chunk 0), f = j*128 + p
    xts, xt0s = [], []
    for b in range(B):
        xr = x[b].T.reshape(NM, 2, P, NCH, 512)       # [m,u,p,c,t]
        xr = xr.transpose(2, 3, 0, 1, 4).reshape(P, NCH, 16 * 512)
        xts.append(_to_f8(xr[:, 1:] * XSC))
        x0 = x[b][:512].T.reshape(16, P, 512).transpose(1, 0, 2)  # [p,j,t]
        xt0s.append(np.ascontiguousarray(x0.reshape(P, 16 * 512)).astype(BF))

    in_maps = []
    for core in range(8):
        b, g = divmod(core, KV)
        wq_g = Wq[:, g * GQ * HD:(g + 1) * GQ * HD]
        wk_g = Wk[:, g * HD:(g + 1) * HD]
        wv_g = Wv[:, g * HD:(g + 1) * HD]
        in_maps.append({
            "xt": xts[b], "xt0": xt0s[b],
            "wq": _to_f8(_pack_dr(wq_g) * WSC),
            "wk": _to_f8(_pack_dr(wk_g) * WSC),
            "wv": _to_f8(_pack_dr(wv_g) * WSC),
            "wqb": _pack_bf(wq_g), "wkb": _pack_bf(wk_g), "wvb": _pack_bf(wv_g),
            "wo": np.ascontiguousarray(
                Wo[g * GQ * HD:(g + 1) * GQ * HD, :].reshape(GQ, P, DIM)
                .transpose(1, 0, 2)).astype(BF),
            "cosq": cq, "sinq": sq, "cosk": ck, "sink": sk,
            "mneg": mneg, "rsw": rsw,
        })

    res = run_bass_kernel_spmd(nc, in_maps, list(range(8)))
    out = np.zeros((B, S, DIM), dtype=np.float32)
    for core in range(8):
        out[core // KV] += res.results[core]["po"].astype(np.float32)
    out *= 1.0 / VSC
    return out


# revision 31
# speedup vs baseline: 1.1593x; 1.0300x over previous
"""GroupedQueryAttention Trainium2 kernel (8 NeuronCores).

Sharding: (batch b in 0..1) x (kv-head group g in 0..3) -> core 4*b+g.
Each core computes, for its batch, the 4 query heads (4g..4g+3) that share
kv head g, plus the partial output projection through the matching 512-row
slice of Wo.  The host sums the 4 partials per batch.

On-device dataflow is fully "transposed": activations live as [feature,
token] so every matmul contraction sits on the partition axis, and the
softmax probabilities come out directly in the layout the P@V matmul
needs.  The whole computation is chunk-streamed (512 tokens per chunk):
projections -> rmsnorm+rope -> attention -> partial out-projection per
chunk, so all engines pipeline across chunks.

fp8 (e4m3, TRN flavor: max +-240) with DoubleRow matmuls is used for the
projections (x, Wq/Wk/Wv pre-packed host-side into contraction-pair
layout) and for P@V plus the softmax denominators (exp writes fp8 probs
directly; the denominator uses the same quantized probs so normalization
is consistent).  Scores (q.k) and the output projection stay bf16.

The causal mask is folded into the score PSUM accumulation as one extra
matmul (identity stationary x precomputed -30000 step tile) so exp
produces exact zeros above the diagonal and no post-exp masking op is
needed.  1/den is computed as exp(-ln(den)) on the scalar engine - both
functions live in the already-loaded activation table set.
"""

import numpy as np
import ml_dtypes

DIM, H, KV, S, B = 2048, 16, 4, 2048, 2
HD = DIM // H          # 128
GQ = H // KV           # 4 query heads per kv head
P = 128                # partitions
NCH = S // 512         # 4 sequence chunks of 512
NM = DIM // 256        # 8 double-row contraction tiles (256 features each)
EPS = 1e-6
BF = ml_dtypes.bfloat16
F8 = ml_dtypes.float8_e4m3   # TRN fp8_e4m3: max +-240

XSC = 16.0             # fp8 scale for x
WSC = 64.0             # fp8 scale for Wq/Wk/Wv
VSC = 32.0             # scale carried by v through P@V (folded out on host)
PSC = 16.0             # prob downscale inside exp (cancels in normalize)

_CACHED = {}


def _build_program():
    import concourse.bass as bass
    import concourse.tile as tile
    from concourse import bacc
    from concourse import mybir
    from concourse.masks import make_identity

    f32 = mybir.dt.float32
    bf16 = mybir.dt.bfloat16
    fp8 = mybir.dt.float8e4
    AF = mybir.ActivationFunctionType
    DR = mybir.MatmulPerfMode.DoubleRow

    nc = bacc.Bacc()
    # all parameters pre-packed on host so every DMA is contiguous per
    # partition
    xt = nc.declare_dram_parameter("xt", [P, NCH - 1, 16 * 512], fp8, isOutput=False)
    xt0 = nc.declare_dram_parameter("xt0", [P, 16 * 512], bf16, isOutput=False)
    wq = nc.declare_dram_parameter("wq", [P, 16, 512], fp8, isOutput=False)
    wk = nc.declare_dram_parameter("wk", [P, 16, HD], fp8, isOutput=False)
    wv = nc.declare_dram_parameter("wv", [P, 16, HD], fp8, isOutput=False)
    wqb = nc.declare_dram_parameter("wqb", [P, 16, 512], bf16, isOutput=False)
    wkb = nc.declare_dram_parameter("wkb", [P, 16, HD], bf16, isOutput=False)
    wvb = nc.declare_dram_parameter("wvb", [P, 16, HD], bf16, isOutput=False)
    wo = nc.declare_dram_parameter("wo", [P, GQ, DIM], bf16, isOutput=False)
    cosq = nc.declare_dram_parameter("cosq", [HD, S], bf16, isOutput=False)
    sinq = nc.declare_dram_parameter("sinq", [HD, S], bf16, isOutput=False)
    cosk = nc.declare_dram_parameter("cosk", [HD, S], bf16, isOutput=False)
    sink = nc.declare_dram_parameter("sink", [HD, S], bf16, isOutput=False)
    mneg = nc.declare_dram_parameter("mneg", [P, 4, 512], bf16, isOutput=False)
    rsw = nc.declare_dram_parameter("rsw", [P, P], bf16, isOutput=False)
    po = nc.declare_dram_parameter("po", [S, DIM], bf16, isOutput=True)

    inv_sqrt_hd = 1.0 / float(np.sqrt(HD))
    nln16 = -float(np.log(PSC))

    with tile.TileContext(nc) as tc:
      with tc.tile_pool(name="const", bufs=1) as const, \
           tc.tile_pool(name="wp", bufs=1) as wp, \
           tc.tile_pool(name="xtp", bufs=1) as xtp, \
           tc.tile_pool(name="actp", bufs=1) as actp, \
           tc.tile_pool(name="hatp", bufs=1) as hatp, \
           tc.tile_pool(name="w2", bufs=2) as w2, \
           tc.tile_pool(name="ptp", bufs=8) as ptp, \
           tc.tile_pool(name="rowp", bufs=3) as rowp, \
           tc.tile_pool(name="ppP", bufs=2, space="PSUM") as ppP, \
           tc.tile_pool(name="psc", bufs=2, space="PSUM") as psc, \
           tc.tile_pool(name="pots", bufs=1, space="PSUM") as pots, \
           tc.tile_pool(name="pden", bufs=1, space="PSUM") as pden:

        ones_sb = const.tile([P, P], bf16)
        nc.vector.memset(ones_sb, 1.0)
        ones8 = const.tile([P, 2, P], fp8)
        nc.vector.memset(ones8, 1.0)
        ident = const.tile([P, P], bf16)
        make_identity(nc, ident)
        rsw_sb = const.tile([P, P], bf16)
        nc.scalar.dma_start(out=rsw_sb, in_=rsw[:, :])
        epsb = const.tile([P, 1], f32)
        nc.vector.memset(epsb, EPS)
        nl16b = const.tile([P, 1], f32)
        nc.vector.memset(nl16b, nln16)
        zerob = const.tile([P, 1], f32)
        nc.vector.memset(zerob, 0.0)

        # input staging.  sync queue, in dependency order: chunk-0 bf16
        # inputs first, then the fp8 pair-packed inputs for chunks 1-3.
        # gpsimd queue: the late-needed weights.
        wkb_sb = wp.tile([P, 16, HD], bf16)
        xt0_sb = xtp.tile([P, 16, 512], bf16)
        wqb_sb = wp.tile([P, 16, 512], bf16)
        wvb_sb = wp.tile([P, 16, HD], bf16)
        wk_sb = wp.tile([P, 16, HD], fp8)
        xt_sb = xtp.tile([P, NCH - 1, 16, 512], fp8)
        wq_sb = wp.tile([P, 16, 512], fp8)
        wv_sb = wp.tile([P, 16, HD], fp8)
        xt8_ap = xt.ap().rearrange("p c (m t) -> p c m t", m=16)
        nc.sync.dma_start(out=wkb_sb, in_=wkb.ap())
        nc.sync.dma_start(out=xt0_sb, in_=xt0.ap().rearrange("p (m t) -> p m t", m=16))
        # gate the second DMA wave on the chunk-0 inputs having landed so
        # the startup-critical transfers get the full HBM bandwidth
        scr = const.tile([P, 16], bf16)
        nc.gpsimd.tensor_copy(scr, xt0_sb[:, 0, 0:16])
        nc.gpsimd.dma_start(out=xt_sb[:, 0], in_=xt8_ap[:, 0])
        nc.gpsimd.dma_start(out=wq_sb, in_=wq.ap())
        nc.gpsimd.dma_start(out=wk_sb, in_=wk.ap())
        nc.gpsimd.dma_start(out=wv_sb, in_=wv.ap())
        nc.gpsimd.dma_start(out=wqb_sb, in_=wqb.ap())
        nc.gpsimd.dma_start(out=wvb_sb, in_=wvb.ap())
        cs_tabs = {"cosq": cosq, "sinq": sinq, "cosk": cosk, "sink": sink}
        cs_sb = {}
        for nm in cs_tabs:
            cs_sb[nm] = wp.tile([P, S], bf16, tag=f"cs_{nm}", name=f"cs_{nm}")

        def fetch_cs(c):
            cl = slice(c * 512, (c + 1) * 512)
            for nm, t in cs_tabs.items():
                nc.scalar.dma_start(out=cs_sb[nm][:, cl], in_=t[:, cl])

        fetch_cs(0)
        mneg_sb = wp.tile([P, 4, 512], bf16)
        nc.scalar.dma_start(out=mneg_sb, in_=mneg.ap())
        wo_sb = wp.tile([P, GQ, DIM], bf16)

        # per-chunk rings for the pre-norm projections (slot 0-3 = q, 4 = k)
        src_t = {}
        vT = actp.tile([P, S], bf16, tag="vT")
        ot_t = {}
        dn_t = {}
        onorm_t = {}
        v_nat = actp.tile([P, 16, HD], fp8, tag="vnat")
        v_nat0 = actp.tile([P, 4, HD], bf16, tag="vnat0")
        qhat = [hatp.tile([P, S], bf16, tag=f"qhat{h}", name=f"qhat{h}")
                for h in range(GQ)]
        khat = hatp.tile([P, S], bf16, tag="khat")

        def emit_p1(c, slots=(4, 0, 1, 2, 3, 5)):
            # fp8 DoubleRow projections (transposed outputs)
            # slot order: k first (needed by every score tile), then q0-3, v
            sl = slice(c * 512, (c + 1) * 512)
            for slot in slots:
                ps = ppP.tile([P, 512], f32, tag="pp", name=f"proj_{c}_{slot}")
                if c == 0:
                    # chunk 0 in bf16: early tokens attend to few keys, so
                    # fp8 noise does not average out there
                    for j in range(16):
                        if slot < 4:
                            lhs = wqb_sb[:, j, slot * HD:(slot + 1) * HD]
                        elif slot == 4:
                            lhs = wkb_sb[:, j, :]
                        else:
                            lhs = wvb_sb[:, j, :]
                        nc.tensor.matmul(ps, lhs, xt0_sb[:, j, :],
                                         start=(j == 0), stop=(j == 15))
                else:
                    for m in range(NM):
                        if slot < 4:
                            lhs = wq_sb[:, 2 * m:2 * m + 2, slot * HD:(slot + 1) * HD]
                        elif slot == 4:
                            lhs = wk_sb[:, 2 * m:2 * m + 2, :]
                        else:
                            lhs = wv_sb[:, 2 * m:2 * m + 2, :]
                        nc.tensor.matmul(ps, lhs,
                                         xt_sb[:, c - 1, 2 * m:2 * m + 2, :],
                                         start=(m == 0), stop=(m == NM - 1),
                                         perf_mode=DR)
                vsc = VSC if c == 0 else (VSC / (XSC * WSC))
                if slot < 5:
                    st = actp.tile([P, 512], bf16, tag=f"src{slot}", bufs=2,
                                   name=f"src_{c}_{slot}")
                    src_t[(c, slot)] = st
                    nc.vector.tensor_copy(st, ps)
                else:
                    # store vT = VSC*v (chunk-0 PSUM is v, fp8 is XSC*WSC*v)
                    nc.scalar.activation(vT[:, sl], ps, AF.Copy, scale=vsc)
            if 5 not in slots:
                return
            # v -> natural layout [key_local, hd] for this chunk's 4 key tiles
            for jj in range(4):
                j = 4 * c + jj
                tp = ppP.tile([P, HD], bf16, tag="pp", name=f"vtr_{j}")
                nc.tensor.transpose(tp, vT[:, j * HD:(j + 1) * HD], ident)
                nc.scalar.copy(v_nat[:, j, :], tp)
                if c == 0:
                    nc.vector.tensor_copy(v_nat0[:, j, :], tp)

        def emit_p2(c):
            # rmsnorm (pre-gain) + rope, chunk-granular.  All 5 tensors'
            # square-sums are computed first so the PE block of matmuls runs
            # with its inputs already resolved; all 5 Ln's precede all 5
            # Exp's so the activation table swaps twice per chunk, not per
            # tensor; rope's rotation leaves PSUM immediately via an ACT
            # copy so the PSUM ring never waits on the DVE stream.
            sl = slice(c * 512, (c + 1) * 512)
            # late-needed inputs are issued from here so the early DMAs get
            # the full HBM bandwidth at kernel start
            if c + 1 < NCH:
                fetch_cs(c + 1)
            if c == 0:
                nc.scalar.dma_start(out=xt_sb[:, 1], in_=xt8_ap[:, 1])
                nc.scalar.dma_start(out=wo_sb, in_=wo.ap())
            elif c == 1:
                nc.scalar.dma_start(out=xt_sb[:, 2], in_=xt8_ap[:, 2])
            srcs = {t: src_t.pop((c, t)) for t in (4, 0, 1, 2, 3)}
            sqbs, rots, lnbs, t3s = {}, {}, {}, {}
            for t in (4, 0, 1, 2, 3):
                sqb = w2.tile([P, 512], bf16, tag="sqb", bufs=5)
                nc.vector.tensor_mul(sqb, srcs[t], srcs[t])
                sqbs[t] = sqb
            # --- Ln block: rmsnorm logs + previous chunk's denominator logs
            # share one activation-table residency ---
            for t in (4, 0, 1, 2, 3):
                ssq = ppP.tile([P, 512], f32, tag="pp", name=f"ssq_{c}_{t}")
                nc.tensor.matmul(ssq, ones_sb, sqbs[t], start=True, stop=True)
                lnb = w2.tile([P, 512], bf16, tag="lnb", bufs=5)
                nc.scalar.activation(lnb, ssq, AF.Ln, bias=epsb, scale=1.0 / HD)
                lnbs[t] = lnb
                rotp = ppP.tile([P, 512], f32, tag="pp", name=f"rot_{c}_{t}")
                nc.tensor.matmul(rotp, rsw_sb, srcs[t], start=True, stop=True)
                rot = w2.tile([P, 512], bf16, tag="rot", bufs=5)
                nc.scalar.copy(rot, rotp)
                rots[t] = rot
            for t in (4, 0, 1, 2, 3):
                cosT = cs_sb["cosq" if t < 4 else "cosk"][:, sl]
                sinT = cs_sb["sinq" if t < 4 else "sink"][:, sl]
                t1 = w2.tile([P, 512], bf16, tag="t1")
                nc.vector.tensor_mul(t1, srcs[t], cosT)
                t2 = w2.tile([P, 512], bf16, tag="t2")
                nc.vector.tensor_mul(t2, rots[t], sinT)
                t3 = w2.tile([P, 512], bf16, tag="t3", bufs=5)
                nc.vector.tensor_add(t3, t1, t2)
                t3s[t] = t3
            # --- Exp block ---
            rsbs = {}
            for t in (4, 0, 1, 2, 3):
                rsb = w2.tile([P, 512], bf16, tag="rsb", bufs=2)
                nc.scalar.activation(rsb, lnbs[t], AF.Exp, bias=zerob,
                                     scale=-0.5)
                rsbs[t] = rsb
            for t in (4, 0, 1, 2, 3):
                dst = (qhat[t] if t < 4 else khat)[:, sl]
                nc.vector.tensor_mul(dst, t3s[t], rsbs[t])


        def emit_p3(c, p5of=None):
            # attention, head-outer.  Out-projection (i, n) groups of chunk
            # `p5of` (whose onorm settled a chunk ago) are interleaved
            # between score tiles: they fill the PE while the scalar engine
            # drains the exp queue, and never wait on the normalize chain.
            sl = slice(c * 512, (c + 1) * 512)
            npr = 2 * c + 2
            po_groups = ([(4 * p5of + i2, n) for i2 in range(4)
                          for n in range(NCH)] if p5of is not None else [])
            pgi = [0]

            def drain_po(k=1):
                while k > 0 and pgi[0] < len(po_groups):
                    emit_po_group(*po_groups[pgi[0]])
                    pgi[0] += 1
                    k -= 1

            for h in range(GQ):
                pts = []
                for pr in range(npr):
                    sc = psc.tile([P, 2, 512], f32, tag="sc",
                                  name=f"sc_{c}_{h}_{pr}")
                    for u in range(2):
                        j = 2 * pr + u
                        if j >= 4 * c:
                            # diagonal key tile: fold causal mask into the
                            # PSUM accumulation (-30000 above the diagonal)
                            nc.tensor.matmul(sc[:, u, :], ident,
                                             mneg_sb[:, j - 4 * c, :],
                                             start=True, stop=False)
                        nc.tensor.matmul(sc[:, u, :],
                                         khat[:, j * P:(j + 1) * P],
                                         qhat[h][:, sl],
                                         start=(j < 4 * c), stop=True)
                    if c == 0:
                        pt = ptp.tile([P, 2, 512], bf16, tag="pt0",
                                      name=f"pt_{c}_{h}_{pr}", bufs=2)
                    else:
                        pt = ptp.tile([P, 2, 512], fp8, tag="pt",
                                      name=f"pt_{c}_{h}_{pr}")
                    nc.scalar.activation(pt, sc, AF.Exp,
                                         bias=nl16b, scale=inv_sqrt_hd)
                    pts.append(pt)
                    if pr % 2 == 1:
                        drain_po()
                ot = pots.tile([P, 512], f32, tag="ot", name=f"ot_{c}_{h}")
                dn = pden.tile([P, 512], f32, tag="dn", name=f"dn_{c}_{h}")
                if c == 0:
                    # chunk 0 stays bf16 through P@V / denominator as well
                    for pr in range(npr):
                        for u in range(2):
                            j = 2 * pr + u
                            nc.tensor.matmul(ot, v_nat0[:, j, :],
                                             pts[pr][:, u, :],
                                             start=(j == 0), stop=(j == 3))
                    for pr in range(npr):
                        for u in range(2):
                            j = 2 * pr + u
                            nc.tensor.matmul(dn, ones_sb, pts[pr][:, u, :],
                                             start=(j == 0), stop=(j == 3))
                else:
                    for pr in range(npr):
                        nc.tensor.matmul(ot, v_nat[:, 2 * pr:2 * pr + 2, :],
                                         pts[pr], start=(pr == 0),
                                         stop=(pr == npr - 1), perf_mode=DR)
                    for pr in range(npr):
                        nc.tensor.matmul(dn, ones8, pts[pr], start=(pr == 0),
                                         stop=(pr == npr - 1), perf_mode=DR)
                rec = w2.tile([P, 512], f32, tag="rec", bufs=2)
                nc.vector.reciprocal(rec, dn)
                ono = hatp.tile([P, 512], bf16, tag=f"onorm{h}", bufs=2,
                                name=f"onorm_{c}_{h}")
                onorm_t[(c, h)] = ono
                nc.vector.tensor_mul(ono, ot, rec)
                drain_po()
            drain_po(len(po_groups))

        def emit_po_group(i, n):
            # one 128-token x 512-col group of the partial out-projection
            ci, i2 = divmod(i, 4)
            isl = slice(i2 * P, (i2 + 1) * P)
            osl = slice(i * P, (i + 1) * P)
            pops = ppP.tile([P, 512], f32, tag="pp", name=f"po_{i}_{n}")
            for h in range(GQ):
                nc.tensor.matmul(pops, onorm_t[(ci, h)][:, isl],
                                 wo_sb[:, h, n * 512:(n + 1) * 512],
                                 start=(h == 0), stop=(h == GQ - 1))
            row = rowp.tile([P, 512], bf16, tag="row")
            nc.vector.tensor_copy(row, pops)
            nc.sync.dma_start(out=po[osl, n * 512:(n + 1) * 512],
                              in_=row)

        # emission order tuned for the in-order engine queues: the chunk-0
        # k-projection (smallest DMA footprint) first, then the fp8 chunk-1
        # projections (their inputs are small and land early), then the rest
        # of chunk 0.  P5(c) is emitted interleaved into P3(c+1) a chunk
        # late so the normalize chain's latency is fully hidden.
        emit_p1(0, slots=(4,))
        emit_p1(1)
        emit_p1(0, slots=(0, 1, 2, 3, 5))
        emit_p2(0)
        emit_p3(0)
        emit_p1(2)
        emit_p2(1)
        emit_p3(1, p5of=0)
        emit_p1(3)
        emit_p2(2)
        emit_p3(2, p5of=1)
        emit_p2(3)
        emit_p3(3, p5of=2)
        for i in range(12, 16):
            for n in range(NCH):
                emit_po_group(i, n)
    nc.compile()
    return nc


def _causal_ok(mask):
    m = np.asarray(mask).reshape(S, S)
    tri = np.tril(np.ones((S, S), dtype=bool))
    return bool(np.all(m[tri] == 0.0) and np.all(m[~tri] <= -1e8))


def _reference_fallback(x, Wq, Wk, Wv, Wo, qg, kg, cos, sin, mask):
    x64 = np.asarray(x, dtype=np.float32)
    q = (x64 @ Wq).reshape(B, S, H, HD).transpose(0, 2, 1, 3)
    k = (x64 @ Wk).reshape(B, S, KV, HD).transpose(0, 2, 1, 3)
    v = (x64 @ Wv).reshape(B, S, KV, HD).transpose(0, 2, 1, 3)

    def rms(t, g):
        r = np.sqrt(np.mean(t * t, axis=-1, keepdims=True) + EPS)
        return g * (t / r)

    q, k = rms(q, qg), rms(k, kg)

    def rot(t):
        return np.concatenate([-t[..., HD // 2:], t[..., :HD // 2]], axis=-1)

    c = cos[None, None, :, :]
    s = sin[None, None, :, :]
    q = q * c + rot(q) * s
    k = k * c + rot(k) * s
    k = np.repeat(k, GQ, axis=1)
    v = np.repeat(v, GQ, axis=1)
    sc = np.einsum('bhqd,bhkd->bhqk', q, k) / np.sqrt(HD) + np.asarray(mask).reshape(1, 1, S, S)
    sc = sc - sc.max(axis=-1, keepdims=True)
    e = np.exp(sc)
    a = e / e.sum(axis=-1, keepdims=True)
    o = np.einsum('bhqk,bhkd->bhqd', a, v)
    o = o.transpose(0, 2, 1, 3).reshape(B, S, H * HD)
    return (o @ Wo).astype(np.float32)


def _to_f8(a):
    return np.ascontiguousarray(np.clip(a, -240.0, 240.0)).astype(F8)


def _pack_dr(w):
    """[DIM, cols] -> [P, 2*m, cols] with feature f = m*256 + u*128 + p."""
    cols = w.shape[1]
    return np.ascontiguousarray(
        w.reshape(NM, 2, P, cols).transpose(2, 0, 1, 3).reshape(P, 2 * NM, cols))


def _pack_bf(w):
    """[DIM, cols] -> [P, j, cols] bf16 with feature f = j*128 + p."""
    cols = w.shape[1]
    return np.ascontiguousarray(
        w.reshape(16, P, cols).transpose(1, 0, 2)).astype(BF)


def kernel(x, Wq, Wk, Wv, Wo, qg, kg, cos, sin, mask, **_unused):
    x = np.asarray(x, dtype=np.float32)
    Wq, Wk, Wv, Wo = (np.asarray(a, dtype=np.float32) for a in (Wq, Wk, Wv, Wo))
    qg, kg = np.asarray(qg, np.float32), np.asarray(kg, np.float32)
    cos, sin = np.asarray(cos, np.float32), np.asarray(sin, np.float32)
    if not _causal_ok(mask):
        return _reference_fallback(x, Wq, Wk, Wv, Wo, qg, kg, cos, sin, mask)

    from concourse.bass_utils import run_bass_kernel_spmd

    if "nc" not in _CACHED:
        _CACHED["nc"] = _build_program()
    nc = _CACHED["nc"]

    cosT = np.ascontiguousarray(cos.T)  # [HD, S]
    sinT = np.ascontiguousarray(sin.T)

    # rope via halves: out[:64] = x[:64]*cos[:64] + x[64:]*st[:64]
    #                  out[64:] = x[64:]*cos[64:] + x[:64]*st[64:]
    # (rot(x)[:64] = -x[64:], rot(x)[64:] = x[:64]; gains fold into tables)
    def tables(g):
        ct = cosT * g[:, None]
        st = np.empty_like(sinT)
        st[:64] = -sinT[:64] * g[64:, None]
        st[64:] = sinT[64:] * g[:64, None]
        return ct.astype(BF), st.astype(BF)

    cq, sq = tables(qg)
    ck, sk = tables(kg)

    rsw = np.zeros((P, P), dtype=np.float32)
    for i in range(P):
        rsw[i, (i + 64) % P] = 1.0
    rsw = rsw.astype(BF)

    cols = np.arange(512)[None, :]
    rows = np.arange(P)[:, None]
    mneg = np.stack([
        np.where(cols - P * a >= rows, 0.0, -30000.0) for a in range(4)
    ]).transpose(1, 0, 2).astype(BF)          # [P, 4, 512]
    mneg = np.ascontiguousarray(mneg)

    # xt: [P, chunk-1, m, u, t] fp8 (chunks 1-3), f = m*256 + u*128 + p
    # xt0: [P, j, t] bf16 (chunk 0), f = j*128 + p
    xts, xt0s = [], []
    for b in range(B):
        xr = x[b].T.reshape(NM, 2, P, NCH, 512)       # [m,u,p,c,t]
        xr = xr.transpose(2, 3, 0, 1, 4).reshape(P, NCH, 16 * 512)
        xts.append(_to_f8(xr[:, 1:] * XSC))
        x0 = x[b][:512].T.reshape(16, P, 512).transpose(1, 0, 2)  # [p,j,t]
        xt0s.append(np.ascontiguousarray(x0.reshape(P, 16 * 512)).astype(BF))

    in_maps = []
    for core in range(8):
        b, g = divmod(core, KV)
        wq_g = Wq[:, g * GQ * HD:(g + 1) * GQ * HD]
        wk_g = Wk[:, g * HD:(g + 1) * HD]
        wv_g = Wv[:, g * HD:(g + 1) * HD]
        in_maps.append({
            "xt": xts[b], "xt0": xt0s[b],
            "wq": _to_f8(_pack_dr(wq_g) * WSC),
            "wk": _to_f8(_pack_dr(wk_g) * WSC),
            "wv": _to_f8(_pack_dr(wv_g) * WSC),
            "wqb": _pack_bf(wq_g), "wkb": _pack_bf(wk_g), "wvb": _pack_bf(wv_g),
            "wo": np.ascontiguousarray(
                Wo[g * GQ * HD:(g + 1) * GQ * HD, :].reshape(GQ, P, DIM)
                .transpose(1, 0, 2)).astype(BF),
            "cosq": cq, "sinq": sq, "cosk": ck, "sink": sk,
            "mneg": mneg, "rsw": rsw,
        })

    res = run_bass_kernel_spmd(nc, in_maps, list(range(8)))
    out = np.zeros((B, S, DIM), dtype=np.float32)
    for core in range(8):
        out[core // KV] += res.results[core]["po"].astype(np.float32)
    out *= 1.0 / VSC
    return out
